# revision 25
# baseline (speedup 1.0000x reference)
"""CrossTransformerBlock Trainium2 kernel v2.

Problem: B=8, C=512, T=1024 tokens (32x32), 8 heads x 64 head-dim.
Data-parallel: one batch element per NeuronCore, 8 cores, no collectives.

Differences vs v1 (119us):
  * QKV projections in fp8e4 DoubleRow (contract 256 rows/matmul at 0.5
    cyc/row): x-hat prescale and the folded-LN weights quantized to e4m3.
    PE proj cost 25.5us -> 7.7us.
  * Softmax exp split across ScalarE (native Exp -> e5m2) and VectorE
    (one-pass "bitcast exp": uint8 = max(s' + 26, 0) reinterpreted as
    e5m2; the score PSUM arrives pre-scaled by 4/(8 ln2) via a host-side
    fold into Wq, so the uint8 code IS 4*log2(C*exp(s/8))). ScalarE's
    bias (-5.94) makes both paths produce the same C*exp(s/8) scale.
    e5m2's 32-binade range covers the heavy score tails that overflow
    e4m3.
  * es in fp8e5, V in fp8e4: AV DoubleRow with mixed operand dtypes.
  * Normalize broadcast via a PE ones-matmul into PSUM instead of the
    DRAM round-trip (kills 32 DMAs + latency).
  * GpSimd (Pool) takes all SBUF-only elementwise work (bf16 stats
    copy, squares, x-hat prescale) - it cannot touch PSUM.
  * Out-projection stays bf16 (PE has slack; buys precision margin).
  * Merged DMAs (x loads 4, weight loads 8, stores 8).

Schedule: qh-major group order (qh, b); kT needs both token-halves of
its chunk, qT only the qh half, so the drip through attention kc-slots
{2,4,6} carries k(m,h0/h1) + q(m,h0) during the qh0 sweep, q(m,h1) and
the half-0 output projection during the qh1 sweep. Normalize deferred
one group as in v1.
"""

import math
import os
from collections import deque

import numpy as np
import ml_dtypes

import concourse.bass as bass
import concourse.mybir as mybir
import concourse.tile as tile

P = 128
C = 512          # embed channels
T = 1024         # tokens (32*32)
NH = 8           # heads
HD = 64          # head dim
B = 8            # batch == n_cores
EPS = 1e-5
NCH = C // P     # 4 channel chunks
NKC = T // P     # 8 token chunks

F32 = mybir.dt.float32
BF16 = mybir.dt.bfloat16
FP8E4 = mybir.dt.float8e4
FP8E5 = mybir.dt.float8e5
U8 = mybir.dt.uint8
AF = mybir.ActivationFunctionType
OP = mybir.AluOpType
DR = mybir.MatmulPerfMode.DoubleRow
BF16NP = ml_dtypes.bfloat16
E4NP = ml_dtypes.float8_e4m3
VDP = HD + 8     # fp8 V row padded so the DoubleRow K-half step is 16B-aligned

CS = 4.0 / (8.0 * math.log(2.0))   # 0.7213: folded into Wq so PSUM scores
                                   # are already in e5m2-code units
B_U8 = 26.0                        # uint8-exp bias (cancels in softmax)
BETA_A = -5.8543                   # ScalarE exp bias matching the uint8 path
                                   # (HW rounds the uint8 convert; CoreSim truncates
                                   # and reads ~4% low on the DVE-chunk weights)
# per-group kc -> exp engine: 'A' ScalarE, 'D' VectorE (8 groups x 8 kc)
_pat_env = os.environ.get("KERNEL_EXP_PAT", "AAADADAA")
if "," in _pat_env:
    EXP_PAT = tuple(_pat_env.split(","))
    assert len(EXP_PAT) == 8
else:
    EXP_PAT = tuple(_pat_env for g in range(8))
DRIP_MODE = os.environ.get("KERNEL_DRIP", "slots")  # slots | front
KC_RECIP = int(os.environ.get("KERNEL_KC_RECIP", "0"))
KC_MULS = int(os.environ.get("KERNEL_KC_MULS", "2"))
DRIP_SLOTS = tuple(int(c) for c in os.environ.get("KERNEL_DRIP_SLOTS", "246"))

_NC_CACHE = {}
LAST_RESULTS = None  # BassKernelResults of the most recent kernel() call


def build_nc():
    if "nc" in _NC_CACHE:
        return _NC_CACHE["nc"]
    nc = bass.Bass()

    xkv_d = nc.declare_dram_parameter("xkv", [C, T], F32, isOutput=False)
    xq_d = nc.declare_dram_parameter("xq", [C, T], F32, isOutput=False)
    w8_d = {}
    we_d = {}
    for name in ("wq", "wk", "wv"):
        w8_d[name] = nc.declare_dram_parameter(f"{name}8", [P, 2, 2, C], FP8E4,
                                               isOutput=False)
        we_d[name] = nc.declare_dram_parameter(f"{name}e", [1, 2, C], FP8E4,
                                               isOutput=False)
    wp_d = nc.declare_dram_parameter("wp", [P, NCH, C], BF16, isOutput=False)
    bp_d = nc.declare_dram_parameter("bp", [C], F32, isOutput=False)
    out_d = nc.declare_dram_parameter("out", [C, T], F32, isOutput=True)

    with tile.TileContext(nc) as tc, \
         tc.tile_pool(name="consts", bufs=1) as consts, \
         tc.tile_pool(name="wpool", bufs=1) as wpool, \
         tc.tile_pool(name="xpool", bufs=1) as xpool, \
         tc.tile_pool(name="stat", bufs=2) as statp, \
         tc.tile_pool(name="ev8", bufs=4) as ev8p, \
         tc.tile_pool(name="actp", bufs=1) as actp, \
         tc.tile_pool(name="spool", bufs=4) as spool, \
         tc.tile_pool(name="npool", bufs=4) as npool, \
         tc.tile_pool(name="opool", bufs=4) as opool, \
         tc.tile_pool(name="dscr", bufs=4, space="DRAM") as dscr, \
         tc.tile_pool(name="ps_s", bufs=2, space="PSUM") as ps_s_pool, \
         tc.tile_pool(name="ps_y", bufs=4, space="PSUM") as ps_y:

        # ---------- constants ----------
        ones_col = consts.tile([P, 1], BF16, tag="ones_col", name="ones_col")
        nc.gpsimd.memset(ones_col, 1.0)
        ones_r64 = consts.tile([1, HD], BF16, tag="ones_r64", name="ones_r64")
        nc.gpsimd.memset(ones_r64, 1.0)
        eps_t = consts.tile([2, 1], F32, tag="eps", name="eps")
        nc.gpsimd.memset(eps_t, EPS)
        # dummy Ln pulls the natural_log_exp ACT table load into the DMA wait
        warm = consts.tile([1, 1], F32, tag="warm", name="warm")
        nc.scalar.activation(out=warm, in_=eps_t[0:1], func=AF.Ln,
                             bias=eps_t[0:1], scale=1.0)
        beta_t = consts.tile([P, 1], F32, tag="beta", name="beta")
        nc.gpsimd.memset(beta_t, BETA_A)
        zeros_t = consts.tile([P, T], BF16, tag="zeros", name="zeros")
        nc.gpsimd.memset(zeros_t, 0.0)
        bp_sb = consts.tile([P, NCH], F32, tag="bp", name="bp")
        nc.sync.dma_start(out=bp_sb, in_=bp_d[:].rearrange("(o p) -> p o", p=P))
        # PE p-state warm-up: ~4us of dummy matmuls during the x-load wait so
        # the LN-stats matmuls run at full clock
        ps_w = ps_y.tile([P, 512], F32, tag="ps_y", name="ps_warm")
        for i in range(12):
            nc.tensor.matmul(ps_w, lhsT=zeros_t[:, 0:P], rhs=zeros_t[:, 0:512],
                             start=(i == 0), stop=(i == 11))

        # ---------- activations + weights, interleaved by first need ------
        # SP DMAs serialize (issue + device transfer), so each weight load is
        # slotted right after the x half whose derived x-hat it meets
        xkv = xpool.tile([P, NCH, T], F32, tag="xkv", name="xkv")
        xq = xpool.tile([P, NCH, T], F32, tag="xq", name="xq")
        w8 = {}
        we = {}

        def load_x(xt, xd, half):
            hs = slice(half * 512, (half + 1) * 512)
            nc.sync.dma_start(
                out=xt[:, :, hs],
                in_=xd[:, hs].rearrange("(o p) t -> p o t", p=P))

        def load_w(name):
            w8[name] = wpool.tile([P, 2, 2, C], FP8E4, tag=f"{name}8",
                                  name=f"{name}8")
            nc.sync.dma_start(out=w8[name], in_=w8_d[name][:])
            we[name] = wpool.tile([1, 2, C], FP8E4, tag=f"{name}e",
                                  name=f"{name}e")
            nc.sync.dma_start(out=we[name], in_=we_d[name][:])

        load_x(xkv, xkv_d, 0)
        load_x(xkv, xkv_d, 1)
        load_w("wk")
        load_x(xq, xq_d, 0)
        load_w("wv")
        load_w("wq")
        load_x(xq, xq_d, 1)
        wp_sb = wpool.tile([P, NCH, C], BF16, tag="wpm", name="wpm")
        nc.sync.dma_start(out=wp_sb, in_=wp_d[:])

        # ---------- phase 1: LN stats + fp8 x-hat, per (tensor, half) -------
        tensors = (("kv", xkv), ("q", xq))
        xh, xe = {}, {}
        for name, x in tensors:
            xh[name] = actp.tile([P, NCH, T], FP8E4, tag=f"xh_{name}",
                                 name=f"xh_{name}")
            xe[name] = actp.tile([1, 2, T], FP8E4, tag=f"xe_{name}",
                                 name=f"xe_{name}")
            nc.gpsimd.memset(xe[name][0:1, 1, :], 1.0)

        def emit_stats(name, x, half):
            hs = slice(half * 512, (half + 1) * 512)
            xb = statp.tile([P, NCH, 512], BF16, tag="xb", name=f"xb_{name}{half}")
            if half == 1:
                nc.gpsimd.tensor_copy(out=xb, in_=x[:, :, hs])
            elif name == "kv":
                nc.scalar.mul(out=xb, in_=x[:, :, hs], mul=1.0)
            else:
                nc.vector.tensor_scalar_mul(xb, x[:, :, hs], 1.0)
            sq = statp.tile([P, NCH, 512], BF16, tag="sq", name=f"sq_{name}{half}")
            nc.vector.tensor_mul(out=sq, in0=xb, in1=xb)
            ps_sum = ps_s_pool.tile([1, 512], F32, tag="ps_s", name="ps_sum")
            for o in range(NCH):
                nc.tensor.matmul(ps_sum, lhsT=ones_col, rhs=xb[:, o, :],
                                 start=(o == 0), stop=(o == NCH - 1))
            ps_sq = ps_s_pool.tile([1, 512], F32, tag="ps_s", name="ps_sq")
            for o in range(NCH):
                nc.tensor.matmul(ps_sq, lhsT=ones_col, rhs=sq[:, o, :],
                                 start=(o == 0), stop=(o == NCH - 1))
            tmp = npool.tile([1, 512], F32, tag="stmp", name=f"t_{name}{half}")
            rrow = npool.tile([1, 512], BF16, tag="rrow", name=f"r_{name}{half}")
            # mu^2 = Square(ps_sum/C); var = ps_sq/C - mu^2 (PSUM read direct)
            nc.scalar.activation(out=tmp, in_=ps_sum, func=AF.Square,
                                 scale=1.0 / C)
            nc.vector.scalar_tensor_tensor(
                out=tmp, in0=ps_sq, scalar=1.0 / C,
                in1=tmp, op0=OP.mult, op1=OP.subtract)
            # r = rsqrt(var+eps) = exp(-0.5*ln(var+eps)), bf16 out
            nc.scalar.activation(out=tmp, in_=tmp, func=AF.Ln,
                                 bias=eps_t[0:1], scale=1.0)
            nc.scalar.activation(out=rrow, in_=tmp, func=AF.Exp, scale=-0.5)
            # xe row 0 = -mu*r = (ps_sum * -1/C) * r  (fp8e4)
            with nc.allow_low_precision(reason="LN shift row in fp8"):
                nc.vector.scalar_tensor_tensor(
                    out=xe[name][0:1, 0, hs], in0=ps_sum, scalar=-1.0 / C,
                    in1=rrow, op0=OP.mult, op1=OP.mult)
            # broadcast r over 128 partitions via PE, evict bf16 for Pool
            ps_rb = ps_s_pool.tile([P, 512], F32, tag="ps_s", name="ps_rb")
            nc.tensor.matmul(ps_rb, lhsT=ones_col[0:1, 0:1]
                             .to_broadcast((1, P)), rhs=rrow,
                             start=True, stop=True)
            rb = statp.tile([P, 512], BF16, tag="rb", name=f"rb_{name}{half}")
            nc.scalar.mul(out=rb, in_=ps_rb, mul=1.0)
            # x-hat fp8 (Pool; SBUF-only operands)
            with nc.allow_low_precision(reason="x-hat in fp8 for DR matmul"):
                for o in range(NCH):
                    nc.gpsimd.tensor_mul(out=xh[name][:, o, hs],
                                         in0=x[:, o, hs], in1=rb)

        # ---------- phase 2 machinery ----------
        # chunk 0 (heads 0,1) stays bf16 channel-major; chunks 1-3 are
        # bounced through DRAM into the fp8 DoubleRow layout
        # [32-block b | slot | j | t]  (channel = 128b + 64*slot + 32*j + p)
        kT = actp.tile([P, T], BF16, tag="kT", name="kT")
        qT = actp.tile([P, T], BF16, tag="qT", name="qT")
        kT8 = actp.tile([P, 2, 2, T], FP8E4, tag="kT8", name="kT8")
        qT8 = actp.tile([P, 2, 2, T], FP8E4, tag="qT8", name="qT8")
        k8scr = dscr.tile([C, T], FP8E4, tag="k8scr", name="k8scr")
        q8scr = dscr.tile([C, T], FP8E4, tag="q8scr", name="q8scr")

        def emit_proj_half(dst, name, m, half, evict_eng):
            # proj chunk m, token half hs, fp8 DR matmuls.  m == 0 evicts to
            # the bf16 channel-major tile; m >= 1 evicts fp8 and bounces
            # through DRAM into the DoubleRow layout.
            ms = slice(m * P, (m + 1) * P)
            hs = slice(half * 512, (half + 1) * 512)
            ps = ps_y.tile([P, 512], F32, tag="ps_y", name="ps_qk")
            for ch in range(2):
                nc.tensor.matmul(ps, lhsT=w8[name][:, :, ch, ms],
                                 rhs=xh["kv" if name != "wq" else "q"]
                                 [:, 2 * ch:2 * ch + 2, hs],
                                 start=(ch == 0), stop=False, perf_mode=DR)
            nc.tensor.matmul(ps, lhsT=we[name][:, :, ms],
                             rhs=xe["kv" if name != "wq" else "q"][:, :, hs],
                             start=False, stop=True, perf_mode=DR)
            if m == 0:
                if evict_eng == "A":
                    nc.scalar.mul(out=dst[:, hs], in_=ps, mul=1.0)
                else:
                    nc.vector.tensor_scalar_mul(dst[:, hs], ps, 1.0)
                return
            tmp8 = ev8p.tile([P, 512], FP8E4, tag="tmp8", name="tmp8")
            if evict_eng == "A":
                nc.scalar.mul(out=tmp8, in_=ps, mul=1.0)
            else:
                nc.vector.tensor_scalar_mul(tmp8, ps, 1.0)
            scr = k8scr if name == "wk" else q8scr
            d8 = kT8 if name == "wk" else qT8
            nc.sync.dma_start(out=scr[m * P:(m + 1) * P, hs], in_=tmp8)
            nc.sync.dma_start(
                out=d8[32 * m:32 * m + 32, :, :, hs],
                in_=scr[m * P:(m + 1) * P, hs]
                .rearrange("(s j p) t -> p s j t", s=2, j=2))

        # V token-major fp8e4 with ones column at d=64 (softmax denom),
        # kc-PAIR layout for DoubleRow: v_sb[t_p, kc//2, kc%2, h, 0:65]
        v_sb = actp.tile([P, NKC // 2, 2, NH, VDP], FP8E4, tag="v", name="v")
        nc.gpsimd.memset(v_sb, 1.0)

        def emit_v_chunk(mt):
            ps = ps_y.tile([P, C], F32, tag="ps_y", name="ps_v")
            ts_ = slice(mt * P, (mt + 1) * P)
            for ch in range(2):
                nc.tensor.matmul(ps, lhsT=xh["kv"][:, 2 * ch:2 * ch + 2, ts_],
                                 rhs=w8["wv"][:, :, ch, :],
                                 start=(ch == 0), stop=False, perf_mode=DR)
            nc.tensor.matmul(ps, lhsT=xe["kv"][:, :, ts_], rhs=we["wv"],
                             start=False, stop=True, perf_mode=DR)
            _vev = os.environ.get("KERNEL_VEV", "AD")
            if _vev[mt % len(_vev)] == "A":
                nc.scalar.mul(out=v_sb[:, mt // 2, mt % 2, :, 0:HD],
                              in_=ps.rearrange("p (h d) -> p h d", h=NH), mul=1.0)
            else:
                nc.vector.tensor_scalar_mul(
                    v_sb[:, mt // 2, mt % 2, :, 0:HD],
                    ps.rearrange("p (h d) -> p h d", h=NH), 1.0)

        # emit phase-1 + prelude projections; kv side first
        for half in range(2):
            emit_stats("kv", xkv, half)
        for half in range(2):
            emit_proj_half(kT, "wk", 0, half, "D")
        for half in range(2):
            emit_stats("q", xq, half)
        emit_proj_half(qT, "wq", 0, 0, "D")
        for mt in range(NKC):
            emit_v_chunk(mt)
        emit_proj_half(qT, "wq", 0, 1, "D")

        # ---------- phase 3: attention ----------
        yT = actp.tile([P, NCH, T], BF16, tag="yT", name="yT")

        def emit_norm_recip(ps_ys):
            invd = npool.tile([1, 2, 512], BF16, tag="invd", name="invd")
            with nc.allow_low_precision(reason="softmax denom in bf16"):
                for hi in range(2):
                    nc.vector.reciprocal(out=invd[0:1, hi, :],
                                         in_=ps_ys[hi][HD:HD + 1, :])
            return invd

        def emit_norm_bcast(invd):
            # broadcast invd across 64 partitions: DRAM bounce, re-read with
            # a 0-stride partition AP (HW only allows one PSUM input per op)
            drow = dscr.tile([1, 1024], BF16, tag="drow", name="drow")
            nc.sync.dma_start(out=drow, in_=invd.rearrange("p h n -> p (h n)"))
            sb_b = npool.tile([HD, 2, 512], BF16, tag="sb_b", name="sb_b")
            nc.sync.dma_start(out=sb_b,
                              in_=drow[0:1, :].to_broadcast((HD, 1024))
                              .rearrange("p (h n) -> p h n", h=2))
            return sb_b

        def emit_norm_muls(sb_b, ps_ys, b, hs):
            for hi in range(2):
                base = HD * hi
                nc.vector.tensor_mul(out=yT[base:base + HD, b, hs],
                                     in0=ps_ys[hi][0:HD, :],
                                     in1=sb_b[:, hi, :])

        def emit_out_proj(m, half):
            ms = slice(m * P, (m + 1) * P)
            hs = slice(half * 512, (half + 1) * 512)
            ps = ps_y.tile([P, 512], F32, tag="ps_y", name="ps_p")
            for k in range(NCH):
                nc.tensor.matmul(ps, lhsT=wp_sb[:, k, ms], rhs=yT[:, k, hs],
                                 start=(k == 0), stop=(k == NCH - 1))
            ot = opool.tile([P, 512], F32, tag="ot", name="ot")
            nc.vector.scalar_tensor_tensor(
                out=ot, in0=ps, scalar=bp_sb[:, m:m + 1],
                in1=xkv[:, m, hs], op0=OP.add, op1=OP.add)
            nc.sync.dma_start(out=out_v[:, m, hs], in_=ot)

        out_v = out_d[:].rearrange("(o p) t -> p o t", p=P)

        # m=1 prefetched before attention (bounce latency); m=2,3 dripped
        for half in range(2):
            emit_proj_half(kT, "wk", 1, half, "D")
        emit_proj_half(qT, "wq", 1, 0, "D")
        work_q0 = deque()
        for m in range(2, NCH):
            work_q0.append(("proj", kT, "wk", m, 0, "D"))
            work_q0.append(("proj", kT, "wk", m, 1, "D"))
            work_q0.append(("proj", qT, "wq", m, 0, "D"))
        work_q1 = deque()
        for m in range(1, NCH):
            work_q1.append(("proj", qT, "wq", m, 1, "D"))
        for m in range(NCH):
            work_q1.append(("out", m, 0))

        def do_work(q):
            if not q:
                return
            item = q.popleft()
            if item[0] == "proj":
                _, dst, name, m, half, eng = item
                emit_proj_half(dst, name, m, half, eng)
            else:
                _, m, half = item
                emit_out_proj(m, half)

        if DRIP_MODE == "front":
            while work_q0:
                do_work(work_q0)
            while work_q1 and work_q1[0][0] == "proj":
                do_work(work_q1)

        pending = None  # deferred normalization (one group)
        for qh in range(2):
            work = work_q0 if qh == 0 else work_q1
            for b in range(NH // 2):
                hs = slice(qh * 512, (qh + 1) * 512)
                ps_ys = [ps_y.tile([HD + 1, 512], F32, tag="ps_y", name="ps_av")
                         for _ in range(2)]  # [head A, head B]

                def emit_av(kcp, last):
                    for hi in range(2):
                        nc.tensor.matmul(
                            ps_ys[hi],
                            lhsT=v_sb[:, kcp, :, 2 * b + hi, 0:HD + 1],
                            rhs=es_pairs[kcp][:, :, 512 * hi:512 * (hi + 1)],
                            start=(kcp == 0), stop=last, perf_mode=DR)

                gi = 4 * qh + b
                pat = EXP_PAT[gi]
                es_pairs = {}
                for kc in range(NKC):
                    ks = slice(kc * P, (kc + 1) * P)
                    kcp, j = divmod(kc, 2)
                    ps_s = ps_s_pool.tile([P, T], F32, tag="ps_s", name="ps_s")
                    if b == 0:
                        nc.tensor.matmul(ps_s[:, 0:512], lhsT=kT[0:HD, ks],
                                         rhs=qT[0:HD, hs],
                                         start=True, stop=True)
                        nc.tensor.matmul(ps_s[:, 512:1024], lhsT=kT[HD:P, ks],
                                         rhs=qT[HD:P, hs],
                                         start=True, stop=True)
                    else:
                        for sl in range(2):
                            nc.tensor.matmul(
                                ps_s[:, 512 * sl:512 * (sl + 1)],
                                lhsT=kT8[32 * b:32 * b + 32, sl, :, ks],
                                rhs=qT8[32 * b:32 * b + 32, sl, :, hs],
                                start=True, stop=True, perf_mode=DR,
                                tile_position=(32 * b, 0))
                    if j == 0:
                        es_pairs[kcp] = spool.tile([P, 2, T], FP8E5, tag="es",
                                                   name="es")
                    if pat[kc] == "A":
                        nc.scalar.activation(out=es_pairs[kcp][:, j, :],
                                             in_=ps_s, func=AF.Exp,
                                             scale=0.125 / CS,
                                             bias=beta_t[:, 0:1])
                    else:
                        # uint8 = max(s' + 26, 0); bitcast e5m2 == C*exp(s/8)
                        nc.vector.scalar_tensor_tensor(
                            out=es_pairs[kcp][:, j, :].bitcast(U8),
                            in0=ps_s, scalar=B_U8, in1=zeros_t,
                            op0=OP.add, op1=OP.max)
                    if kc == KC_MULS and pending is not None:
                        emit_norm_muls(sbb_p, *pending)
                        pending = None
                    if kc in DRIP_SLOTS:
                        do_work(work)
                    if j == 0 and kcp >= 1:
                        emit_av(kcp - 1, last=False)
                emit_av(NKC // 2 - 1, last=True)
                if qh == 1 and b == NH // 2 - 1:
                    if pending is not None:
                        emit_norm_muls(sbb_p, *pending)
                    # final group: skip the DRAM bounce (its latency would sit
                    # in the drain) - PE ones-matmul broadcast into a free
                    # ps_s slot, evicted to SBUF by the now-idle ScalarE
                    invd = emit_norm_recip(ps_ys)
                    ps_b = ps_s_pool.tile([P, T], F32, tag="ps_s", name="ps_bf")
                    for hi in range(2):
                        nc.tensor.matmul(ps_b[0:HD, 512 * hi:512 * (hi + 1)],
                                         lhsT=ones_r64, rhs=invd[0:1, hi, :],
                                         start=True, stop=True)
                    sbb = npool.tile([HD, 2, 512], BF16, tag="sb_b",
                                     name="sb_bf")
                    nc.scalar.mul(out=sbb,
                                  in_=ps_b[0:HD, :].rearrange(
                                      "p (h n) -> p h n", h=2), mul=1.0)
                    emit_norm_muls(sbb, ps_ys, b, hs)
                    pending = None
                else:
                    sbb_p = emit_norm_bcast(emit_norm_recip(ps_ys))
                    pending = (ps_ys, b, hs)
        while work_q1:
            do_work(work_q1)
        for m in range(NCH):
            emit_out_proj(m, 1)

    if not int(os.environ.get("KERNEL_NO_LEGALIZE", "0")):
        _legalize_waits(nc)
    _NC_CACHE["nc"] = nc
    return nc


def _legalize_waits(nc):
    """walrus in this container rejects instructions with >1 sync-wait
    command ("Too many sync wait commands").  Split extra waits onto
    same-engine NoOp carrier instructions inserted just before."""
    n = 0
    for f in nc.m.functions:
        for blk in f.blocks:
            new_insts = []
            for inst in blk.instructions:
                si = inst.sync_info
                if si is not None and si.on_wait and len(si.on_wait) > 1:
                    for w in si.on_wait[:-1]:
                        n += 1
                        nop = mybir.InstNoOp(name=f"WNOP-{n}", ins=[], outs=[])
                        nop.engine = inst.engine
                        nop.sync_info = mybir.SyncInfo(on_wait=[w], on_update=[])
                        new_insts.append(nop)
                    inst.sync_info = mybir.SyncInfo(
                        on_wait=[si.on_wait[-1]], on_update=si.on_update)
                new_insts.append(inst)
            blk.instructions = new_insts


def _fold_w(W, bias, ln_w, ln_b, scale=1.0):
    """Augmented weight [C+2, C]: rows 0..C-1 = diag(ln_w) @ W,
    row C = colsum(diag(ln_w) @ W), row C+1 = ln_b @ W + bias."""
    W = np.asarray(W, np.float64) * scale
    bias = np.asarray(bias, np.float64) * scale
    ln_w = np.asarray(ln_w, np.float64)
    ln_b = np.asarray(ln_b, np.float64)
    Wm = ln_w[:, None] * W
    u = Wm.sum(axis=0)
    b2 = ln_b @ W + bias
    return np.concatenate([Wm, u[None], b2[None]], axis=0)


def _pack_w8(Waug):
    """[C+2, C] -> main [128, 2, 2, C] (row c = ch*256 + j*128 + p) and
    ext [1, 2, C] (rows C, C+1), both e4m3."""
    main = Waug[:C].reshape(2, 2, P, C).transpose(2, 1, 0, C and 3)
    # reshape(2(ch), 2(j), 128(p), C) -> [p, j, ch, C]
    main = Waug[:C].reshape(2, 2, P, C).transpose(2, 1, 0, 3)
    ext = Waug[C:C + 2].reshape(1, 2, C)
    return (np.ascontiguousarray(main).astype(E4NP),
            np.ascontiguousarray(ext).astype(E4NP))


def make_in_maps(q, kv, ln_kv_w, ln_kv_b, ln_q_w, ln_q_b,
                 Wk, bk, Wq, bq, Wv, bv, Wp, bp):
    q = np.asarray(q, np.float32)
    kv = np.asarray(kv, np.float32)
    wq8, wqe = _pack_w8(_fold_w(Wq, bq, ln_q_w, ln_q_b, scale=CS))
    wk8, wke = _pack_w8(_fold_w(Wk, bk, ln_kv_w, ln_kv_b))
    wv8, wve = _pack_w8(_fold_w(Wv, bv, ln_kv_w, ln_kv_b))
    wp_bf = np.ascontiguousarray(
        np.asarray(Wp, np.float64).reshape(NCH, P, C).transpose(1, 0, 2)
    ).astype(BF16NP)
    bp_f = np.asarray(bp, np.float32)
    in_maps = []
    for b_ in range(B):
        in_maps.append({
            "xkv": np.ascontiguousarray(kv[b_].reshape(C, T)),
            "xq": np.ascontiguousarray(q[b_].reshape(C, T)),
            "wq8": wq8, "wqe": wqe,
            "wk8": wk8, "wke": wke,
            "wv8": wv8, "wve": wve,
            "wp": wp_bf,
            "bp": bp_f,
        })
    return in_maps


def kernel(**inputs):
    global LAST_RESULTS
    from concourse.bass_utils import run_bass_kernel_spmd

    nc = build_nc()
    in_maps = make_in_maps(**inputs)
    trace = bool(int(os.environ.get("KERNEL_TRACE", "0")))
    res = run_bass_kernel_spmd(nc, in_maps, list(range(B)), trace=trace)
    LAST_RESULTS = res
    out = np.stack([np.asarray(res.results[i]["out"], np.float32)
                    for i in range(B)], axis=0)
    H = W_ = 32
    return out.reshape(B, C, H, W_)


# revision 26
# speedup vs baseline: 1.0048x; 1.0048x over previous
"""CrossTransformerBlock Trainium2 kernel v2.

Problem: B=8, C=512, T=1024 tokens (32x32), 8 heads x 64 head-dim.
Data-parallel: one batch element per NeuronCore, 8 cores, no collectives.

Differences vs v1 (119us):
  * QKV projections in fp8e4 DoubleRow (contract 256 rows/matmul at 0.5
    cyc/row): x-hat prescale and the folded-LN weights quantized to e4m3.
    PE proj cost 25.5us -> 7.7us.
  * Softmax exp split across ScalarE (native Exp -> e5m2) and VectorE
    (one-pass "bitcast exp": uint8 = max(s' + 26, 0) reinterpreted as
    e5m2; the score PSUM arrives pre-scaled by 4/(8 ln2) via a host-side
    fold into Wq, so the uint8 code IS 4*log2(C*exp(s/8))). ScalarE's
    bias (-5.94) makes both paths produce the same C*exp(s/8) scale.
    e5m2's 32-binade range covers the heavy score tails that overflow
    e4m3.
  * es in fp8e5, V in fp8e4: AV DoubleRow with mixed operand dtypes.
  * Normalize broadcast via a PE ones-matmul into PSUM instead of the
    DRAM round-trip (kills 32 DMAs + latency).
  * GpSimd (Pool) takes all SBUF-only elementwise work (bf16 stats
    copy, squares, x-hat prescale) - it cannot touch PSUM.
  * Out-projection stays bf16 (PE has slack; buys precision margin).
  * Merged DMAs (x loads 4, weight loads 8, stores 8).

Schedule: qh-major group order (qh, b); kT needs both token-halves of
its chunk, qT only the qh half, so the drip through attention kc-slots
{2,4,6} carries k(m,h0/h1) + q(m,h0) during the qh0 sweep, q(m,h1) and
the half-0 output projection during the qh1 sweep. Normalize deferred
one group as in v1.
"""

import math
import os
from collections import deque

import numpy as np
import ml_dtypes

import concourse.bass as bass
import concourse.mybir as mybir
import concourse.tile as tile

P = 128
C = 512          # embed channels
T = 1024         # tokens (32*32)
NH = 8           # heads
HD = 64          # head dim
B = 8            # batch == n_cores
EPS = 1e-5
NCH = C // P     # 4 channel chunks
NKC = T // P     # 8 token chunks

F32 = mybir.dt.float32
BF16 = mybir.dt.bfloat16
FP8E4 = mybir.dt.float8e4
FP8E5 = mybir.dt.float8e5
U8 = mybir.dt.uint8
AF = mybir.ActivationFunctionType
OP = mybir.AluOpType
DR = mybir.MatmulPerfMode.DoubleRow
BF16NP = ml_dtypes.bfloat16
E4NP = ml_dtypes.float8_e4m3
VDP = HD + 8     # fp8 V row padded so the DoubleRow K-half step is 16B-aligned

CS = 4.0 / (8.0 * math.log(2.0))   # 0.7213: folded into Wq so PSUM scores
                                   # are already in e5m2-code units
B_U8 = 26.0                        # uint8-exp bias (cancels in softmax)
BETA_A = -5.8543                   # ScalarE exp bias matching the uint8 path
                                   # (HW rounds the uint8 convert; CoreSim truncates
                                   # and reads ~4% low on the DVE-chunk weights)
# per-group kc -> exp engine: 'A' ScalarE, 'D' VectorE (8 groups x 8 kc)
_pat_env = os.environ.get("KERNEL_EXP_PAT", "AAADADAA")
if "," in _pat_env:
    EXP_PAT = tuple(_pat_env.split(","))
    assert len(EXP_PAT) == 8
else:
    EXP_PAT = tuple(_pat_env for g in range(8))
DRIP_MODE = os.environ.get("KERNEL_DRIP", "slots")  # slots | front
KC_RECIP = int(os.environ.get("KERNEL_KC_RECIP", "0"))
KC_MULS = int(os.environ.get("KERNEL_KC_MULS", "2"))
DRIP_SLOTS = tuple(int(c) for c in os.environ.get("KERNEL_DRIP_SLOTS", "246"))

_NC_CACHE = {}
LAST_RESULTS = None  # BassKernelResults of the most recent kernel() call


def build_nc():
    if "nc" in _NC_CACHE:
        return _NC_CACHE["nc"]
    nc = bass.Bass()

    xkv_d = nc.declare_dram_parameter("xkv", [C, T], F32, isOutput=False)
    xq_d = nc.declare_dram_parameter("xq", [C, T], F32, isOutput=False)
    w8_d = {}
    we_d = {}
    for name in ("wq", "wk", "wv"):
        w8_d[name] = nc.declare_dram_parameter(f"{name}8", [P, 2, 2, C], FP8E4,
                                               isOutput=False)
        we_d[name] = nc.declare_dram_parameter(f"{name}e", [1, 2, C], FP8E4,
                                               isOutput=False)
    wp_d = nc.declare_dram_parameter("wp", [P, NCH, C], BF16, isOutput=False)
    bp_d = nc.declare_dram_parameter("bp", [C], F32, isOutput=False)
    out_d = nc.declare_dram_parameter("out", [C, T], F32, isOutput=True)

    with tile.TileContext(nc) as tc, \
         tc.tile_pool(name="consts", bufs=1) as consts, \
         tc.tile_pool(name="wpool", bufs=1) as wpool, \
         tc.tile_pool(name="xpool", bufs=1) as xpool, \
         tc.tile_pool(name="stat", bufs=2) as statp, \
         tc.tile_pool(name="ev8", bufs=4) as ev8p, \
         tc.tile_pool(name="actp", bufs=1) as actp, \
         tc.tile_pool(name="spool", bufs=4) as spool, \
         tc.tile_pool(name="npool", bufs=4) as npool, \
         tc.tile_pool(name="opool", bufs=4) as opool, \
         tc.tile_pool(name="dscr", bufs=4, space="DRAM") as dscr, \
         tc.tile_pool(name="ps_s", bufs=2, space="PSUM") as ps_s_pool, \
         tc.tile_pool(name="ps_y", bufs=4, space="PSUM") as ps_y:

        # ---------- constants ----------
        ones_col = consts.tile([P, 1], BF16, tag="ones_col", name="ones_col")
        nc.gpsimd.memset(ones_col, 1.0)
        ones_r64 = consts.tile([1, HD], BF16, tag="ones_r64", name="ones_r64")
        nc.gpsimd.memset(ones_r64, 1.0)
        eps_t = consts.tile([2, 1], F32, tag="eps", name="eps")
        nc.gpsimd.memset(eps_t, EPS)
        # dummy Ln pulls the natural_log_exp ACT table load into the DMA wait
        warm = consts.tile([1, 1], F32, tag="warm", name="warm")
        nc.scalar.activation(out=warm, in_=eps_t[0:1], func=AF.Ln,
                             bias=eps_t[0:1], scale=1.0)
        beta_t = consts.tile([P, 1], F32, tag="beta", name="beta")
        nc.gpsimd.memset(beta_t, BETA_A)
        zeros_t = consts.tile([P, T], BF16, tag="zeros", name="zeros")
        nc.gpsimd.memset(zeros_t, 0.0)
        bp_sb = consts.tile([P, NCH], F32, tag="bp", name="bp")
        # PE p-state warm-up: ~4us of dummy matmuls during the x-load wait so
        # the LN-stats matmuls run at full clock
        ps_w = ps_y.tile([P, 512], F32, tag="ps_y", name="ps_warm")
        for i in range(12):
            nc.tensor.matmul(ps_w, lhsT=zeros_t[:, 0:P], rhs=zeros_t[:, 0:512],
                             start=(i == 0), stop=(i == 11))

        # ---------- activations + weights, interleaved by first need ------
        # SP DMAs serialize (issue + device transfer), so each weight load is
        # slotted right after the x half whose derived x-hat it meets
        xkv = xpool.tile([P, NCH, T], F32, tag="xkv", name="xkv")
        xq = xpool.tile([P, NCH, T], F32, tag="xq", name="xq")
        w8 = {}
        we = {}

        def load_x(xt, xd, half):
            hs = slice(half * 512, (half + 1) * 512)
            nc.sync.dma_start(
                out=xt[:, :, hs],
                in_=xd[:, hs].rearrange("(o p) t -> p o t", p=P))

        def load_w(name):
            w8[name] = wpool.tile([P, 2, 2, C], FP8E4, tag=f"{name}8",
                                  name=f"{name}8")
            nc.sync.dma_start(out=w8[name], in_=w8_d[name][:])
            we[name] = wpool.tile([1, 2, C], FP8E4, tag=f"{name}e",
                                  name=f"{name}e")
            nc.sync.dma_start(out=we[name], in_=we_d[name][:])

        load_x(xkv, xkv_d, 0)
        load_x(xkv, xkv_d, 1)
        load_w("wk")
        load_x(xq, xq_d, 0)
        load_w("wv")
        load_w("wq")
        load_x(xq, xq_d, 1)
        # bp after the critical loads: it is tiny and only needed at out-proj
        nc.sync.dma_start(out=bp_sb, in_=bp_d[:].rearrange("(o p) -> p o", p=P))
        wp_sb = wpool.tile([P, NCH, C], BF16, tag="wpm", name="wpm")
        nc.sync.dma_start(out=wp_sb, in_=wp_d[:])

        # ---------- phase 1: LN stats + fp8 x-hat, per (tensor, half) -------
        tensors = (("kv", xkv), ("q", xq))
        xh, xe = {}, {}
        for name, x in tensors:
            xh[name] = actp.tile([P, NCH, T], FP8E4, tag=f"xh_{name}",
                                 name=f"xh_{name}")
            xe[name] = actp.tile([1, 2, T], FP8E4, tag=f"xe_{name}",
                                 name=f"xe_{name}")
            nc.gpsimd.memset(xe[name][0:1, 1, :], 1.0)

        def emit_stats(name, x, half):
            hs = slice(half * 512, (half + 1) * 512)
            xb = statp.tile([P, NCH, 512], BF16, tag="xb", name=f"xb_{name}{half}")
            if half == 1:
                nc.gpsimd.tensor_copy(out=xb, in_=x[:, :, hs])
            elif name == "kv":
                nc.scalar.mul(out=xb, in_=x[:, :, hs], mul=1.0)
            else:
                nc.vector.tensor_scalar_mul(xb, x[:, :, hs], 1.0)
            sq = statp.tile([P, NCH, 512], BF16, tag="sq", name=f"sq_{name}{half}")
            nc.vector.tensor_mul(out=sq, in0=xb, in1=xb)
            ps_sum = ps_s_pool.tile([1, 512], F32, tag="ps_s", name="ps_sum")
            for o in range(NCH):
                nc.tensor.matmul(ps_sum, lhsT=ones_col, rhs=xb[:, o, :],
                                 start=(o == 0), stop=(o == NCH - 1))
            ps_sq = ps_s_pool.tile([1, 512], F32, tag="ps_s", name="ps_sq")
            for o in range(NCH):
                nc.tensor.matmul(ps_sq, lhsT=ones_col, rhs=sq[:, o, :],
                                 start=(o == 0), stop=(o == NCH - 1))
            tmp = npool.tile([1, 512], F32, tag="stmp", name=f"t_{name}{half}")
            rrow = npool.tile([1, 512], BF16, tag="rrow", name=f"r_{name}{half}")
            # mu^2 = Square(ps_sum/C); var = ps_sq/C - mu^2 (PSUM read direct)
            nc.scalar.activation(out=tmp, in_=ps_sum, func=AF.Square,
                                 scale=1.0 / C)
            nc.vector.scalar_tensor_tensor(
                out=tmp, in0=ps_sq, scalar=1.0 / C,
                in1=tmp, op0=OP.mult, op1=OP.subtract)
            # r = rsqrt(var+eps) = exp(-0.5*ln(var+eps)), bf16 out
            nc.scalar.activation(out=tmp, in_=tmp, func=AF.Ln,
                                 bias=eps_t[0:1], scale=1.0)
            nc.scalar.activation(out=rrow, in_=tmp, func=AF.Exp, scale=-0.5)
            # xe row 0 = -mu*r = (ps_sum * -1/C) * r  (fp8e4)
            with nc.allow_low_precision(reason="LN shift row in fp8"):
                nc.vector.scalar_tensor_tensor(
                    out=xe[name][0:1, 0, hs], in0=ps_sum, scalar=-1.0 / C,
                    in1=rrow, op0=OP.mult, op1=OP.mult)
            # broadcast r over 128 partitions via PE, evict bf16 for Pool
            ps_rb = ps_s_pool.tile([P, 512], F32, tag="ps_s", name="ps_rb")
            nc.tensor.matmul(ps_rb, lhsT=ones_col[0:1, 0:1]
                             .to_broadcast((1, P)), rhs=rrow,
                             start=True, stop=True)
            rb = statp.tile([P, 512], BF16, tag="rb", name=f"rb_{name}{half}")
            nc.scalar.mul(out=rb, in_=ps_rb, mul=1.0)
            # x-hat fp8 (Pool; SBUF-only operands)
            with nc.allow_low_precision(reason="x-hat in fp8 for DR matmul"):
                for o in range(NCH):
                    nc.gpsimd.tensor_mul(out=xh[name][:, o, hs],
                                         in0=x[:, o, hs], in1=rb)

        # ---------- phase 2 machinery ----------
        # chunk 0 (heads 0,1) stays bf16 channel-major; chunks 1-3 are
        # bounced through DRAM into the fp8 DoubleRow layout
        # [32-block b | slot | j | t]  (channel = 128b + 64*slot + 32*j + p)
        kT = actp.tile([P, T], BF16, tag="kT", name="kT")
        qT = actp.tile([P, T], BF16, tag="qT", name="qT")
        kT8 = actp.tile([P, 2, 2, T], FP8E4, tag="kT8", name="kT8")
        qT8 = actp.tile([P, 2, 2, T], FP8E4, tag="qT8", name="qT8")
        k8scr = dscr.tile([C, T], FP8E4, tag="k8scr", name="k8scr")
        q8scr = dscr.tile([C, T], FP8E4, tag="q8scr", name="q8scr")

        def emit_proj_half(dst, name, m, half, evict_eng):
            # proj chunk m, token half hs, fp8 DR matmuls.  m == 0 evicts to
            # the bf16 channel-major tile; m >= 1 evicts fp8 and bounces
            # through DRAM into the DoubleRow layout.
            ms = slice(m * P, (m + 1) * P)
            hs = slice(half * 512, (half + 1) * 512)
            ps = ps_y.tile([P, 512], F32, tag="ps_y", name="ps_qk")
            for ch in range(2):
                nc.tensor.matmul(ps, lhsT=w8[name][:, :, ch, ms],
                                 rhs=xh["kv" if name != "wq" else "q"]
                                 [:, 2 * ch:2 * ch + 2, hs],
                                 start=(ch == 0), stop=False, perf_mode=DR)
            nc.tensor.matmul(ps, lhsT=we[name][:, :, ms],
                             rhs=xe["kv" if name != "wq" else "q"][:, :, hs],
                             start=False, stop=True, perf_mode=DR)
            if m == 0:
                if evict_eng == "A":
                    nc.scalar.mul(out=dst[:, hs], in_=ps, mul=1.0)
                else:
                    nc.vector.tensor_scalar_mul(dst[:, hs], ps, 1.0)
                return
            tmp8 = ev8p.tile([P, 512], FP8E4, tag="tmp8", name="tmp8")
            if evict_eng == "A":
                nc.scalar.mul(out=tmp8, in_=ps, mul=1.0)
            else:
                nc.vector.tensor_scalar_mul(tmp8, ps, 1.0)
            scr = k8scr if name == "wk" else q8scr
            d8 = kT8 if name == "wk" else qT8
            nc.sync.dma_start(out=scr[m * P:(m + 1) * P, hs], in_=tmp8)
            nc.sync.dma_start(
                out=d8[32 * m:32 * m + 32, :, :, hs],
                in_=scr[m * P:(m + 1) * P, hs]
                .rearrange("(s j p) t -> p s j t", s=2, j=2))

        # V token-major fp8e4 with ones column at d=64 (softmax denom),
        # kc-PAIR layout for DoubleRow: v_sb[t_p, kc//2, kc%2, h, 0:65]
        v_sb = actp.tile([P, NKC // 2, 2, NH, VDP], FP8E4, tag="v", name="v")
        nc.gpsimd.memset(v_sb, 1.0)

        def emit_v_chunk(mt):
            ps = ps_y.tile([P, C], F32, tag="ps_y", name="ps_v")
            ts_ = slice(mt * P, (mt + 1) * P)
            for ch in range(2):
                nc.tensor.matmul(ps, lhsT=xh["kv"][:, 2 * ch:2 * ch + 2, ts_],
                                 rhs=w8["wv"][:, :, ch, :],
                                 start=(ch == 0), stop=False, perf_mode=DR)
            nc.tensor.matmul(ps, lhsT=xe["kv"][:, :, ts_], rhs=we["wv"],
                             start=False, stop=True, perf_mode=DR)
            _vev = os.environ.get("KERNEL_VEV", "AD")
            if _vev[mt % len(_vev)] == "A":
                nc.scalar.mul(out=v_sb[:, mt // 2, mt % 2, :, 0:HD],
                              in_=ps.rearrange("p (h d) -> p h d", h=NH), mul=1.0)
            else:
                nc.vector.tensor_scalar_mul(
                    v_sb[:, mt // 2, mt % 2, :, 0:HD],
                    ps.rearrange("p (h d) -> p h d", h=NH), 1.0)

        # emit phase-1 + prelude projections; kv side first
        for half in range(2):
            emit_stats("kv", xkv, half)
        for half in range(2):
            emit_proj_half(kT, "wk", 0, half, "D")
        for half in range(2):
            emit_stats("q", xq, half)
        emit_proj_half(qT, "wq", 0, 0, "D")
        for mt in range(NKC):
            emit_v_chunk(mt)
        emit_proj_half(qT, "wq", 0, 1, "D")

        # ---------- phase 3: attention ----------
        yT = actp.tile([P, NCH, T], BF16, tag="yT", name="yT")

        def emit_norm_recip(ps_ys):
            invd = npool.tile([1, 2, 512], BF16, tag="invd", name="invd")
            with nc.allow_low_precision(reason="softmax denom in bf16"):
                for hi in range(2):
                    nc.vector.reciprocal(out=invd[0:1, hi, :],
                                         in_=ps_ys[hi][HD:HD + 1, :])
            return invd

        def emit_norm_bcast(invd):
            # broadcast invd across 64 partitions: DRAM bounce, re-read with
            # a 0-stride partition AP (HW only allows one PSUM input per op)
            drow = dscr.tile([1, 1024], BF16, tag="drow", name="drow")
            nc.sync.dma_start(out=drow, in_=invd.rearrange("p h n -> p (h n)"))
            sb_b = npool.tile([HD, 2, 512], BF16, tag="sb_b", name="sb_b")
            nc.sync.dma_start(out=sb_b,
                              in_=drow[0:1, :].to_broadcast((HD, 1024))
                              .rearrange("p (h n) -> p h n", h=2))
            return sb_b

        def emit_norm_muls(sb_b, ps_ys, b, hs):
            for hi in range(2):
                base = HD * hi
                nc.vector.tensor_mul(out=yT[base:base + HD, b, hs],
                                     in0=ps_ys[hi][0:HD, :],
                                     in1=sb_b[:, hi, :])

        def emit_out_proj(m, half):
            ms = slice(m * P, (m + 1) * P)
            hs = slice(half * 512, (half + 1) * 512)
            ps = ps_y.tile([P, 512], F32, tag="ps_y", name="ps_p")
            for k in range(NCH):
                nc.tensor.matmul(ps, lhsT=wp_sb[:, k, ms], rhs=yT[:, k, hs],
                                 start=(k == 0), stop=(k == NCH - 1))
            ot = opool.tile([P, 512], F32, tag="ot", name="ot")
            nc.vector.scalar_tensor_tensor(
                out=ot, in0=ps, scalar=bp_sb[:, m:m + 1],
                in1=xkv[:, m, hs], op0=OP.add, op1=OP.add)
            nc.sync.dma_start(out=out_v[:, m, hs], in_=ot)

        out_v = out_d[:].rearrange("(o p) t -> p o t", p=P)

        # m=1 prefetched before attention (bounce latency); m=2,3 dripped
        for half in range(2):
            emit_proj_half(kT, "wk", 1, half, "D")
        emit_proj_half(qT, "wq", 1, 0, "D")
        work_q0 = deque()
        for m in range(2, NCH):
            work_q0.append(("proj", kT, "wk", m, 0, "D"))
            work_q0.append(("proj", kT, "wk", m, 1, "D"))
            work_q0.append(("proj", qT, "wq", m, 0, "D"))
        work_q1 = deque()
        for m in range(1, NCH):
            work_q1.append(("proj", qT, "wq", m, 1, "D"))
        for m in range(NCH):
            work_q1.append(("out", m, 0))

        def do_work(q):
            if not q:
                return
            item = q.popleft()
            if item[0] == "proj":
                _, dst, name, m, half, eng = item
                emit_proj_half(dst, name, m, half, eng)
            else:
                _, m, half = item
                emit_out_proj(m, half)

        if DRIP_MODE == "front":
            while work_q0:
                do_work(work_q0)
            while work_q1 and work_q1[0][0] == "proj":
                do_work(work_q1)

        pending = None  # deferred normalization (one group)
        for qh in range(2):
            work = work_q0 if qh == 0 else work_q1
            for b in range(NH // 2):
                hs = slice(qh * 512, (qh + 1) * 512)
                ps_ys = [ps_y.tile([HD + 1, 512], F32, tag="ps_y", name="ps_av")
                         for _ in range(2)]  # [head A, head B]

                def emit_av(kcp, last):
                    for hi in range(2):
                        nc.tensor.matmul(
                            ps_ys[hi],
                            lhsT=v_sb[:, kcp, :, 2 * b + hi, 0:HD + 1],
                            rhs=es_pairs[kcp][:, :, 512 * hi:512 * (hi + 1)],
                            start=(kcp == 0), stop=last, perf_mode=DR)

                gi = 4 * qh + b
                pat = EXP_PAT[gi]
                es_pairs = {}
                for kc in range(NKC):
                    ks = slice(kc * P, (kc + 1) * P)
                    kcp, j = divmod(kc, 2)
                    ps_s = ps_s_pool.tile([P, T], F32, tag="ps_s", name="ps_s")
                    if b == 0:
                        nc.tensor.matmul(ps_s[:, 0:512], lhsT=kT[0:HD, ks],
                                         rhs=qT[0:HD, hs],
                                         start=True, stop=True)
                        nc.tensor.matmul(ps_s[:, 512:1024], lhsT=kT[HD:P, ks],
                                         rhs=qT[HD:P, hs],
                                         start=True, stop=True)
                    else:
                        for sl in range(2):
                            nc.tensor.matmul(
                                ps_s[:, 512 * sl:512 * (sl + 1)],
                                lhsT=kT8[32 * b:32 * b + 32, sl, :, ks],
                                rhs=qT8[32 * b:32 * b + 32, sl, :, hs],
                                start=True, stop=True, perf_mode=DR,
                                tile_position=(32 * b, 0))
                    if j == 0:
                        es_pairs[kcp] = spool.tile([P, 2, T], FP8E5, tag="es",
                                                   name="es")
                    if pat[kc] == "A":
                        nc.scalar.activation(out=es_pairs[kcp][:, j, :],
                                             in_=ps_s, func=AF.Exp,
                                             scale=0.125 / CS,
                                             bias=beta_t[:, 0:1])
                    else:
                        # uint8 = max(s' + 26, 0); bitcast e5m2 == C*exp(s/8)
                        nc.vector.scalar_tensor_tensor(
                            out=es_pairs[kcp][:, j, :].bitcast(U8),
                            in0=ps_s, scalar=B_U8, in1=zeros_t,
                            op0=OP.add, op1=OP.max)
                    if kc == KC_MULS and pending is not None:
                        emit_norm_muls(sbb_p, *pending)
                        pending = None
                    if kc in DRIP_SLOTS:
                        do_work(work)
                    if j == 0 and kcp >= 1:
                        emit_av(kcp - 1, last=False)
                emit_av(NKC // 2 - 1, last=True)
                if qh == 1 and b == NH // 2 - 1:
                    if pending is not None:
                        emit_norm_muls(sbb_p, *pending)
                    # final group: skip the DRAM bounce (its latency would sit
                    # in the drain) - PE ones-matmul broadcast into a free
                    # ps_s slot, evicted to SBUF by the now-idle ScalarE
                    invd = emit_norm_recip(ps_ys)
                    ps_b = ps_s_pool.tile([P, T], F32, tag="ps_s", name="ps_bf")
                    for hi in range(2):
                        nc.tensor.matmul(ps_b[0:HD, 512 * hi:512 * (hi + 1)],
                                         lhsT=ones_r64, rhs=invd[0:1, hi, :],
                                         start=True, stop=True)
                    sbb = npool.tile([HD, 2, 512], BF16, tag="sb_b",
                                     name="sb_bf")
                    nc.scalar.mul(out=sbb,
                                  in_=ps_b[0:HD, :].rearrange(
                                      "p (h n) -> p h n", h=2), mul=1.0)
                    emit_norm_muls(sbb, ps_ys, b, hs)
                    pending = None
                else:
                    sbb_p = emit_norm_bcast(emit_norm_recip(ps_ys))
                    pending = (ps_ys, b, hs)
        while work_q1:
            do_work(work_q1)
        for m in range(NCH):
            emit_out_proj(m, 1)

    if not int(os.environ.get("KERNEL_NO_LEGALIZE", "0")):
        _legalize_waits(nc)
    _NC_CACHE["nc"] = nc
    return nc


def _legalize_waits(nc):
    """walrus in this container rejects instructions with >1 sync-wait
    command ("Too many sync wait commands").  Split extra waits onto
    same-engine NoOp carrier instructions inserted just before."""
    n = 0
    for f in nc.m.functions:
        for blk in f.blocks:
            new_insts = []
            for inst in blk.instructions:
                si = inst.sync_info
                if si is not None and si.on_wait and len(si.on_wait) > 1:
                    for w in si.on_wait[:-1]:
                        n += 1
                        nop = mybir.InstNoOp(name=f"WNOP-{n}", ins=[], outs=[])
                        nop.engine = inst.engine
                        nop.sync_info = mybir.SyncInfo(on_wait=[w], on_update=[])
                        new_insts.append(nop)
                    inst.sync_info = mybir.SyncInfo(
                        on_wait=[si.on_wait[-1]], on_update=si.on_update)
                new_insts.append(inst)
            blk.instructions = new_insts


def _fold_w(W, bias, ln_w, ln_b, scale=1.0):
    """Augmented weight [C+2, C]: rows 0..C-1 = diag(ln_w) @ W,
    row C = colsum(diag(ln_w) @ W), row C+1 = ln_b @ W + bias."""
    W = np.asarray(W, np.float64) * scale
    bias = np.asarray(bias, np.float64) * scale
    ln_w = np.asarray(ln_w, np.float64)
    ln_b = np.asarray(ln_b, np.float64)
    Wm = ln_w[:, None] * W
    u = Wm.sum(axis=0)
    b2 = ln_b @ W + bias
    return np.concatenate([Wm, u[None], b2[None]], axis=0)


def _pack_w8(Waug):
    """[C+2, C] -> main [128, 2, 2, C] (row c = ch*256 + j*128 + p) and
    ext [1, 2, C] (rows C, C+1), both e4m3."""
    main = Waug[:C].reshape(2, 2, P, C).transpose(2, 1, 0, C and 3)
    # reshape(2(ch), 2(j), 128(p), C) -> [p, j, ch, C]
    main = Waug[:C].reshape(2, 2, P, C).transpose(2, 1, 0, 3)
    ext = Waug[C:C + 2].reshape(1, 2, C)
    return (np.ascontiguousarray(main).astype(E4NP),
            np.ascontiguousarray(ext).astype(E4NP))


def make_in_maps(q, kv, ln_kv_w, ln_kv_b, ln_q_w, ln_q_b,
                 Wk, bk, Wq, bq, Wv, bv, Wp, bp):
    q = np.asarray(q, np.float32)
    kv = np.asarray(kv, np.float32)
    wq8, wqe = _pack_w8(_fold_w(Wq, bq, ln_q_w, ln_q_b, scale=CS))
    wk8, wke = _pack_w8(_fold_w(Wk, bk, ln_kv_w, ln_kv_b))
    wv8, wve = _pack_w8(_fold_w(Wv, bv, ln_kv_w, ln_kv_b))
    wp_bf = np.ascontiguousarray(
        np.asarray(Wp, np.float64).reshape(NCH, P, C).transpose(1, 0, 2)
    ).astype(BF16NP)
    bp_f = np.asarray(bp, np.float32)
    in_maps = []
    for b_ in range(B):
        in_maps.append({
            "xkv": np.ascontiguousarray(kv[b_].reshape(C, T)),
            "xq": np.ascontiguousarray(q[b_].reshape(C, T)),
            "wq8": wq8, "wqe": wqe,
            "wk8": wk8, "wke": wke,
            "wv8": wv8, "wve": wve,
            "wp": wp_bf,
            "bp": bp_f,
        })
    return in_maps


def kernel(**inputs):
    global LAST_RESULTS
    from concourse.bass_utils import run_bass_kernel_spmd

    nc = build_nc()
    in_maps = make_in_maps(**inputs)
    trace = bool(int(os.environ.get("KERNEL_TRACE", "0")))
    res = run_bass_kernel_spmd(nc, in_maps, list(range(B)), trace=trace)
    LAST_RESULTS = res
    out = np.stack([np.asarray(res.results[i]["out"], np.float32)
                    for i in range(B)], axis=0)
    H = W_ = 32
    return out.reshape(B, C, H, W_)
